# revision 45
# baseline (speedup 1.0000x reference)
"""Trainium2 Bass kernel for dynamic low-pass filter decomposition, v6.

Module: global-avg-pool -> 1x1 conv -> BN (inference) -> softmax over 3x3
taps gives a per-(sample, group) 3x3 kernel; applied as a reflect-padded
depthwise conv over x; returns (low, x - low).

Sharding: data-parallel over batch n=8 across 8 NeuronCores (1 sample/core).

v6 design (v4 = 87us, v5 = 102us):
  - Partition layout per band: p = 8*(i+1) + g for band row i (0..13),
    halo-above (row r0-1) at partitions 0..7, halo-below (row r0+14) at
    120..127.  A band's 16 rows are CONSECUTIVE DRAM rows, so each band
    loads with ONE flat contiguous descriptor; 14 descriptors round-robin
    over all three DMA queues (sync/scalar HW-DGE + gpsimd SW-DGE; each
    queue sustains only ~120GB/s on loads, so spread them).
  - Vertical-tap wiring collapses to epat[di][q, p] = (q == p + 8*di).
  - Pooled row-sums on the PE: per band, 3 x 512-col matmuls with a
    group-mask stationary accumulate row sums into a [8, 1536] PSUM
    tile across all bands (keeps the PE HAM-warm through the load
    phase; no DVE tree).  One DVE tensor_reduce over w + tiny matmul
    chain then produces the 72 softmax logits.
  - conv phase: PSUM->SBUF low copy split scalar/DVE, high sub on DVE,
    stores round-robin over all three queues (~120GB/s each).
"""
import sys
import os

sys.path.insert(0, "/opt/trn_rl_repo")

import numpy as np
import ml_dtypes
from contextlib import ExitStack

import concourse.bass as bass
import concourse.tile as tile
from concourse import bacc, mybir
from concourse.bass_utils import run_bass_kernel_spmd

dt = mybir.dt
f32 = dt.float32
bf16 = dt.bfloat16

KS = 3
GROUP = 8
IC = 64
BN_EPS = 1e-5
N = 8
H = W = 192
CW = 8 * W              # free elems per partition (8 chans x 192 cols)
PAD = 2                 # front/back pad elems (4B alignment + shift room)
BR = 14                 # rows per band
NBANDS = 14             # 14 * 14 = 196 >= 192
CH = 512
ROWSTRIDE = IC * W      # 12288 elems per image row in [r][c][w] layout
BW = PAD + CW + PAD     # band window in SBUF free dim
TSTRIDE = BR * ROWSTRIDE  # 172032 elems per band of DRAM rows

DUMMY_N = 6             # PE warm-up matmuls before the first row-sum


def _band_rows(t):
    """(first output row, n output rows) of band t."""
    r0 = BR * t
    return r0, min(BR, H - r0)


def _build_program():
    nc = bacc.Bacc("TRN2", target_bir_lowering=False, debug=False,
                   num_devices=N)

    x_d = nc.dram_tensor("x", [H, IC, W], bf16, kind="ExternalInput")
    e_d = [nc.dram_tensor(f"epat{di}", [128, 128], bf16,
                          kind="ExternalInput") for di in range(3)]
    hv_d = nc.dram_tensor("hv4", [8, 128], bf16, kind="ExternalInput")
    gma_d = nc.dram_tensor("gmA", [128, 512], bf16, kind="ExternalInput")
    gmb_d = nc.dram_tensor("gmB", [128, 512], bf16, kind="ExternalInput")
    a64_d = nc.dram_tensor("a64", [64, 72], bf16, kind="ExternalInput")
    b_d = nc.dram_tensor("b72", [72, 1], f32, kind="ExternalInput")
    r9_d = nc.dram_tensor("r9", [72, 9], bf16, kind="ExternalInput")
    g_d = nc.dram_tensor("g728", [72, 8], bf16, kind="ExternalInput")
    low_d = nc.dram_tensor("low", [H, IC, W], bf16, kind="ExternalOutput")
    high_d = nc.dram_tensor("high", [H, IC, W], bf16, kind="ExternalOutput")

    xd = x_d.ap().tensor

    def flat_rows_ap(r0, np_):
        """np_ partitions <- contiguous chan-blocks starting at row r0
        (partition stride 1536 = one chan-block; 8 blocks per row)."""
        return bass.AP(xd, r0 * ROWSTRIDE, [[CW, np_], [1, CW]])

    def out_ap(dram, t):
        r0, nr = _band_rows(t)
        return bass.AP(dram.ap().tensor, r0 * ROWSTRIDE,
                       [[ROWSTRIDE, nr], [CW, 8], [1, CW]])

    def out_pair_ap(dram, t):
        """Both bands t, t+1 in one descriptor (iter [i][g][band][w],
        matching an SBUF source [112 part][2 bands][1536])."""
        r0, _ = _band_rows(t)
        return bass.AP(dram.ap().tensor, r0 * ROWSTRIDE,
                       [[ROWSTRIDE, BR], [CW, 8], [TSTRIDE, 2], [1, CW]])

    ENGS = None

    with tile.TileContext(nc) as tc, ExitStack() as ctx:
        ENGS = (nc.sync, nc.scalar, nc.gpsimd)
        cpool = ctx.enter_context(tc.tile_pool(name="consts", bufs=1))
        xpool = ctx.enter_context(tc.tile_pool(name="x", bufs=1))
        wpool = ctx.enter_context(tc.tile_pool(name="w", bufs=1))
        spool = ctx.enter_context(tc.tile_pool(name="stage", bufs=4))

        # ---- tiles ----
        xball = xpool.tile([128, NBANDS * BW], bf16)
        xb = [xball[:, t * BW:(t + 1) * BW] for t in range(NBANDS)]
        xv = xball[:].rearrange("p (t b) -> p t b", b=BW)

        warm_s = wpool.tile([128, 128], bf16, name="warm_s")
        warm_m = wpool.tile([128, 512], bf16, name="warm_m")
        nc.vector.memset(warm_s[:], 0.0)
        nc.vector.memset(warm_m[:], 0.0)

        wps_cm = tc.tile_pool(name="wpsum", bufs=1,
                              space=bass.MemorySpace.PSUM)
        wps = wps_cm.__enter__()
        warm_p = wps.tile([128, 512], f32, tag="warm")
        rs_p = wps.tile([64, 192], f32, tag="rowsum")

        # band 13's unloaded partitions: zero BEFORE its loads (NaN in
        # any streamed partition poisons the conv PSUM column; 0*NaN=NaN)
        nc.vector.memset(xb[NBANDS - 1][96:128, PAD:PAD + CW], 0.0)

        # ---- PE warm-up prefix ----
        for _ in range(DUMMY_N):
            nc.tensor.matmul(warm_p[:], warm_s[:], warm_m[:],
                             start=True, stop=True)

        # ---- consts the load/pool phase needs, first on the gpsimd
        # queue (SWDGE issue is ~600ns/descriptor; these are tiny) ----
        e_s = [cpool.tile([128, 128], bf16, name=f"epat{di}")
               for di in range(3)]
        hv_s = cpool.tile([8, 128], bf16)
        gma_s = cpool.tile([128, 512], bf16)
        gmb_s = cpool.tile([128, 512], bf16)
        a64_s = cpool.tile([64, 72], bf16)
        b_s = cpool.tile([72, 1], f32)
        r9_s = cpool.tile([72, 9], bf16)
        g_s = cpool.tile([72, 8], bf16)
        for s, d in ((b_s, b_d), (gma_s, gma_d), (gmb_s, gmb_d)):
            nc.gpsimd.dma_start(s[:], d.ap())

        # ---- x loads: one contiguous descriptor per band, spread over
        # the 3 queues (~120GB/s each on loads).  Band t window holds
        # DRAM rows r0-1..r0+14 at partitions 0..127
        # (p = 8*(row - (r0-1)) + g). ----
        # reflect-row dups must land BEFORE their band's row-sum matmul
        # (which streams all 128 partitions), so issue them ahead of the
        # band mains on their queues.
        nc.sync.dma_start(xb[0][0:8, PAD:PAD + CW], flat_rows_ap(1, 8))
        nc.scalar.dma_start(xb[NBANDS - 1][88:96, PAD:PAD + CW],
                            flat_rows_ap(H - 2, 8))
        for t in range(NBANDS):
            r0, nr = _band_rows(t)
            eng = ENGS[t % 3]
            if t == 0:
                # rows 0..14 -> partitions 8..128; above-halo (reflect
                # row 1) -> partitions 0..8 via the dup above
                eng.dma_start(xb[0][8:128, PAD:PAD + CW],
                              flat_rows_ap(0, 120))
            elif t < NBANDS - 1:
                eng.dma_start(xb[t][0:128, PAD:PAD + CW],
                              flat_rows_ap(r0 - 1, 128))
            else:
                # rows 181..191 -> partitions 0..88; below-halo
                # (reflect row 190) -> partitions 88..96 via the dup
                eng.dma_start(xb[t][0:88, PAD:PAD + CW],
                              flat_rows_ap(r0 - 1, 88))

        # remaining consts (needed from the weight chain onwards)
        for s, d in ((a64_s, a64_d),
                     (e_s[0], e_d[0]), (e_s[1], e_d[1]), (e_s[2], e_d[2]),
                     (hv_s, hv_d), (r9_s, r9_d), (g_s, g_d)):
            nc.gpsimd.dma_start(s[:], d.ap())

        # band-window pads (zero guard columns for the +-1 shifts)
        nc.vector.memset(xv[:, :, 0:PAD], 0.0)
        nc.vector.memset(xv[:, :, PAD + CW:BW], 0.0)

        # pre-load ACT spline tables off the weight-chain critical path
        exp_dummy = wpool.tile([72, 1], f32)
        nc.scalar.activation(exp_dummy[:], b_s[:],
                             mybir.ActivationFunctionType.Exp)

        # ---- pooled row-sums on the PE: rs[8g+cs, w] accumulates the
        # row sums of x[row, 8g+cs, w] across all bands.  Streaming one
        # 192-col chan-sub block per matmul with a per-cs group mask
        # puts each (g, cs) on its own PSUM partition, so the final
        # w-reduce is a tiny [64,192] op and the logit fold is ONE
        # matmul against a64. ----
        for t in range(NBANDS):
            gm = gma_s if t < NBANDS - 1 else gmb_s
            for cs in range(8):
                nc.tensor.matmul(rs_p[:],
                                 gm[:, 64 * cs:64 * (cs + 1)],
                                 xb[t][:, PAD + W * cs:PAD + W * (cs + 1)],
                                 start=(t == 0 and cs == 0),
                                 stop=(t == NBANDS - 1 and cs == 7))

        # ---- weight generation chain ----
        def warm(k, cols=256):
            for _ in range(k):
                nc.tensor.matmul(warm_p[:, 0:cols], warm_s[:],
                                 warm_m[:, 0:cols], start=True, stop=True)

        # w-reduce: [64, 192] PSUM -> [64, 1]; then ONE fold matmul
        p64f = wpool.tile([64, 1], f32)
        nc.vector.tensor_reduce(p64f[:], rs_p[:],
                                axis=mybir.AxisListType.X,
                                op=mybir.AluOpType.add)
        p64s = wpool.tile([64, 1], bf16)
        nc.scalar.copy(p64s[:], p64f[:])
        warm(5)
        # logits: lf[oc] = sum_m a64[m, oc] * P64[m],  m = 8g + cs
        lf_p = wps.tile([72, 1], f32, tag="lf")
        nc.tensor.matmul(lf_p[:], a64_s[:], p64s[:])
        warm(4)
        e72 = wpool.tile([72, 1], f32)
        nc.scalar.activation(e72[:], lf_p[:],
                             mybir.ActivationFunctionType.Exp,
                             bias=b_s[:, 0:1], scale=1.0)
        rhsw = wpool.tile([72, 9], bf16)
        nc.vector.tensor_scalar_mul(rhsw[:], r9_s[:], e72[:, 0:1])
        w89_p = wps.tile([8, 9], f32, tag="w89")
        nc.tensor.matmul(w89_p[:], g_s[:], rhsw[:])
        warm(7)
        s8 = wpool.tile([8, 1], f32)
        nc.vector.tensor_reduce(s8[:], w89_p[:],
                                axis=mybir.AxisListType.X,
                                op=mybir.AluOpType.add)
        r8 = wpool.tile([8, 1], f32)
        nc.vector.reciprocal(r8[:], s8[:])
        w89s = wpool.tile([8, 9], bf16)
        nc.vector.tensor_scalar_mul(w89s[:], w89_p[:], r8[:, 0:1])
        wbig_p = wps.tile([128, 9], f32, tag="wbig")
        nc.tensor.matmul(wbig_p[:], hv_s[:], w89s[:])
        warm(16)
        wsc = wpool.tile([128, 9], f32)
        nc.scalar.copy(wsc[:], wbig_p[:])
        wps_cm.__exit__(None, None, None)

        # ---- the 3 block-banded stationaries: S_dj = sum_di E_di *
        # w[g(q), 3*di+dj] (per-partition row scaling; g(q) = q%8).
        # Each S needs 3 serial ops; split the three chains between DVE
        # and gpsimd so S0 (conv band 0's first stationary) lands fast.
        S = [wpool.tile([128, 128], bf16, name=f"S{dj}") for dj in range(3)]
        for dj in range(3):
            nc.vector.tensor_scalar_mul(S[dj][:], e_s[0][:],
                                        wsc[:, dj:dj + 1])
            for di in (1, 2):
                nc.vector.scalar_tensor_tensor(
                    S[dj][:], e_s[di][:], wsc[:, 3 * di + dj:3 * di + dj + 1],
                    S[dj][:],
                    op0=mybir.AluOpType.mult, op1=mybir.AluOpType.add)
        # edge-fix stationary: with reflection, low[c,0] = S02.x[c,1] +
        # S1.x[c,0] and low[c,191] = S02.x[c,191-1] + S1.x[c,191]
        s02 = wpool.tile([128, 128], bf16, name="s02")
        nc.vector.tensor_tensor(s02[:], S[0][:], S[2][:],
                                op=mybir.AluOpType.add)

        # ---- main loop: one band at a time, acc = 4 PSUM banks
        # (3 x 512 main + edge-fix columns in bank 3) ----
        # conv PSUM: six 1-bank chunk accumulators + one edge tile; each
        # 512-col chunk finishes its 3 dj-passes and drains immediately,
        # so the PE never waits on a whole-band drain chain
        mpool = ctx.enter_context(
            tc.tile_pool(name="mpsum", bufs=6, space=bass.MemorySpace.PSUM))
        epool = ctx.enter_context(
            tc.tile_pool(name="epsum", bufs=1, space=bass.MemorySpace.PSUM))
        # ALL bands' w-edge columns in one pair of 224-col matmuls
        # (xt strides: [band 1540][c 192][e in {1,190} or {0,191}])
        eacc = epool.tile([128, 16 * NBANDS], f32, tag="edge")
        xtc = xv[:, :, PAD:PAD + CW].rearrange("p t (c w) -> p t c w", w=W)
        nc.tensor.matmul(eacc[:], s02[:], xtc[:, :, :, 1:191:189],
                         start=True, stop=False)
        nc.tensor.matmul(eacc[:], S[1][:], xtc[:, :, :, 0:192:191],
                         start=False, stop=True)
        # store queue schedule: 2 stores/band over 3 queues; keep the
        # final bands off the laggy gpsimd software queue so the kernel
        # tail is not gated by it
        sq_rot = [(2, 1), (0, 2), (1, 0)]  # (low_q, high_q) cycling
        for t in range(NBANDS):
            _, nr = _band_rows(t)
            np_ = 8 * nr
            half = t % 2
            lo, hi = half * CW, half * CW + CW
            if half == 0:
                low_st = spool.tile([128, 2 * CW], bf16, tag="low",
                                    name=f"low{t}")
                high_st = spool.tile([128, 2 * CW], bf16, tag="high",
                                     name=f"high{t}")
            for ch in range(3):
                a = mpool.tile([128, 512], f32, tag="acc",
                               name=f"acc{t}_{ch}")
                for j in range(3):
                    off = PAD + CH * ch + j - 1
                    nc.tensor.matmul(a[:], S[j][:], xb[t][:, off:off + CH],
                                     start=(j == 0), stop=(j == 2))
                if ch < 2:
                    nc.scalar.copy(low_st[:, lo + CH * ch:lo + CH * (ch + 1)],
                                   a[:])
                else:
                    nc.vector.tensor_scalar_mul(
                        low_st[:, lo + 1024:hi], a[:], 1.0)
            # output rows of band t live at partitions 8..8+np_
            nc.scalar.copy(
                low_st[:, lo:hi].rearrange("p (c w) -> p c w",
                                           w=W)[:, :, 0:W:W - 1],
                eacc[:, 16 * t:16 * (t + 1)].rearrange(
                    "p (c e) -> p c e", e=2))
            nc.gpsimd.tensor_tensor(high_st[:, lo:lo + 704],
                                    xb[t][:, PAD:PAD + 704],
                                    low_st[:, lo:lo + 704],
                                    op=mybir.AluOpType.subtract)
            nc.vector.tensor_tensor(high_st[:, lo + 704:hi],
                                    xb[t][:, PAD + 704:PAD + CW],
                                    low_st[:, lo + 704:hi],
                                    op=mybir.AluOpType.subtract)
            if t % 2 == 1 and t < NBANDS - 1:
                ql, qh = sq_rot[(t // 2) % 3]
                ENGS[ql].dma_start(out_pair_ap(low_d, t - 1),
                                   low_st[8:120, :])
                ENGS[qh].dma_start(out_pair_ap(high_d, t - 1),
                                   high_st[8:120, :])
            elif t == NBANDS - 2:
                nc.scalar.dma_start(out_ap(low_d, 12),
                                    low_st[8:120, 0:CW])
                nc.sync.dma_start(out_ap(high_d, 12),
                                  high_st[8:120, 0:CW])
            elif t == NBANDS - 1:
                nc.sync.dma_start(out_ap(low_d, 13),
                                  low_st[8:8 + np_, CW:2 * CW])
                nc.scalar.dma_start(out_ap(high_d, 13),
                                    high_st[8:8 + np_, CW:2 * CW])

    nc.compile()
    return nc


def _enable_ldw_opt():
    """Let walrus dedup redundant LDWEIGHTS (same stationary reused by
    consecutive matmuls pipelines at ~N cycles instead of N + ldw)."""
    import concourse.bass_utils as BU
    if getattr(BU, "_ldw_patched", False):
        return
    orig = BU.run_command

    def patched(cmd, *a, **kw):
        cmd = [c.replace("--enable-ldw-opt=false", "--enable-ldw-opt=true")
               if isinstance(c, str) else c for c in cmd]
        return orig(cmd, *a, **kw)

    BU.run_command = patched
    BU._ldw_patched = True


_nc_cache = None


def _get_program():
    global _nc_cache
    if _nc_cache is None:
        # NOTE: _enable_ldw_opt() crashes walrus codegen
        # (visitInstLdweights) with 16-bit stationaries; leave off.
        _nc_cache = _build_program()
    return _nc_cache


def _host_consts(conv_w, bn_gamma, bn_beta, bn_mean, bn_var):
    s_a = bn_gamma / np.sqrt(bn_var + BN_EPS)
    b72 = (bn_beta - bn_mean * s_a).astype(np.float32).reshape(72, 1)
    A = (conv_w * s_a[:, None]) / np.float32(H * W)   # (72, 64)

    # E wiring: band rows r0-1..r0+14 sit at partitions 8*(i_src+1)+g;
    # output row i lives at p = 8*(i+1)+g too (conv output partitions
    # align with input partitions for the DVE subtract), so the source
    # of p for vertical tap di is simply q = p + 8*(di-1).
    epat = [np.zeros((128, 128), np.float32) for _ in range(3)]
    for p in range(8, 120):
        for di in range(3):
            epat[di][p + 8 * (di - 1), p] = 1.0
    epat = [e.astype(ml_dtypes.bfloat16) for e in epat]

    hv4 = (np.arange(8)[:, None] == (np.arange(128)[None, :] % 8)
           ).astype(ml_dtypes.bfloat16)
    # row-sum group masks [128, 8*64]: block cs maps source partition q
    # (valid band row, group g) onto output partition m = 8g + cs
    q = np.arange(128)
    gmA = np.zeros((128, 512), np.float32)
    gmB = np.zeros((128, 512), np.float32)
    for cs in range(8):
        for g in range(8):
            m = 8 * g + cs
            sel = (q % 8 == g)
            gmA[:, 64 * cs + m] = sel & (q >= 8) & (q < 120)
            gmB[:, 64 * cs + m] = sel & (q >= 8) & (q < 88)
    gmA = gmA.astype(ml_dtypes.bfloat16)
    gmB = gmB.astype(ml_dtypes.bfloat16)
    # a64[8g + cs, oc] = A[oc, 8g + cs]  (A cols are channels 8g + cs)
    a64 = np.ascontiguousarray(A.T).astype(ml_dtypes.bfloat16)
    oc = np.arange(72)
    r9 = (oc[:, None] % 9 == np.arange(9)[None, :]
          ).astype(ml_dtypes.bfloat16)
    g728 = (oc[:, None] // 9 == np.arange(8)[None, :]
            ).astype(ml_dtypes.bfloat16)
    return dict(epat0=epat[0], epat1=epat[1], epat2=epat[2], hv4=hv4,
                gmA=gmA, gmB=gmB, a64=a64, b72=b72, r9=r9, g728=g728)


def _prep_inputs(x, conv_w, bn_gamma, bn_beta, bn_mean, bn_var):
    x = np.asarray(x, np.float32)
    consts = _host_consts(np.asarray(conv_w, np.float32),
                          np.asarray(bn_gamma, np.float32),
                          np.asarray(bn_beta, np.float32),
                          np.asarray(bn_mean, np.float32),
                          np.asarray(bn_var, np.float32))
    maps = []
    for i in range(N):
        xr = np.ascontiguousarray(np.transpose(x[i], (1, 0, 2))
                                  ).astype(ml_dtypes.bfloat16)
        maps.append(dict(x=xr, **consts))
    return maps


def _gather(res):
    low = np.stack([np.transpose(np.asarray(res[i]["low"]), (1, 0, 2))
                    for i in range(N)]).astype(np.float32)
    high = np.stack([np.transpose(np.asarray(res[i]["high"]), (1, 0, 2))
                     for i in range(N)]).astype(np.float32)
    return low, high


def kernel(x, conv_w, bn_gamma, bn_beta, bn_mean, bn_var):
    in_maps = _prep_inputs(x, conv_w, bn_gamma, bn_beta, bn_mean, bn_var)
    nc = _get_program()
    res = run_bass_kernel_spmd(nc, in_maps, list(range(N))).results
    return _gather(res)


if __name__ == "__main__":
    rng = np.random.default_rng(0)
    demo = dict(
        x=rng.standard_normal((N, IC, H, W), dtype=np.float32),
        conv_w=rng.standard_normal((72, 64)).astype(np.float32),
        bn_gamma=np.ones(72, np.float32),
        bn_beta=np.zeros(72, np.float32),
        bn_mean=rng.standard_normal(72).astype(np.float32) * 0.1,
        bn_var=rng.uniform(0.5, 1.5, 72).astype(np.float32),
    )
    low, high = kernel(**demo)
    print("ok", low.shape, high.shape)


# revision 46
# speedup vs baseline: 1.0232x; 1.0232x over previous
"""Trainium2 Bass kernel for dynamic low-pass filter decomposition, v6.

Module: global-avg-pool -> 1x1 conv -> BN (inference) -> softmax over 3x3
taps gives a per-(sample, group) 3x3 kernel; applied as a reflect-padded
depthwise conv over x; returns (low, x - low).

Sharding: data-parallel over batch n=8 across 8 NeuronCores (1 sample/core).

v6 design (v4 = 87us, v5 = 102us):
  - Partition layout per band: p = 8*(i+1) + g for band row i (0..13),
    halo-above (row r0-1) at partitions 0..7, halo-below (row r0+14) at
    120..127.  A band's 16 rows are CONSECUTIVE DRAM rows, so each band
    loads with ONE flat contiguous descriptor; 14 descriptors round-robin
    over all three DMA queues (sync/scalar HW-DGE + gpsimd SW-DGE; each
    queue sustains only ~120GB/s on loads, so spread them).
  - Vertical-tap wiring collapses to epat[di][q, p] = (q == p + 8*di).
  - Pooled row-sums on the PE: per band, 3 x 512-col matmuls with a
    group-mask stationary accumulate row sums into a [8, 1536] PSUM
    tile across all bands (keeps the PE HAM-warm through the load
    phase; no DVE tree).  One DVE tensor_reduce over w + tiny matmul
    chain then produces the 72 softmax logits.
  - conv phase: PSUM->SBUF low copy split scalar/DVE, high sub on DVE,
    stores round-robin over all three queues (~120GB/s each).
"""
import sys
import os

sys.path.insert(0, "/opt/trn_rl_repo")

import numpy as np
import ml_dtypes
from contextlib import ExitStack

import concourse.bass as bass
import concourse.tile as tile
from concourse import bacc, mybir
from concourse.bass_utils import run_bass_kernel_spmd

dt = mybir.dt
f32 = dt.float32
bf16 = dt.bfloat16

KS = 3
GROUP = 8
IC = 64
BN_EPS = 1e-5
N = 8
H = W = 192
CW = 8 * W              # free elems per partition (8 chans x 192 cols)
PAD = 2                 # front/back pad elems (4B alignment + shift room)
BR = 14                 # rows per band
NBANDS = 14             # 14 * 14 = 196 >= 192
CH = 512
ROWSTRIDE = IC * W      # 12288 elems per image row in [r][c][w] layout
BW = PAD + CW + PAD     # band window in SBUF free dim
TSTRIDE = BR * ROWSTRIDE  # 172032 elems per band of DRAM rows

DUMMY_N = 6             # PE warm-up matmuls before the first row-sum


def _band_rows(t):
    """(first output row, n output rows) of band t."""
    r0 = BR * t
    return r0, min(BR, H - r0)


def _build_program():
    nc = bacc.Bacc("TRN2", target_bir_lowering=False, debug=False,
                   num_devices=N)

    x_d = nc.dram_tensor("x", [H, IC, W], bf16, kind="ExternalInput")
    e_d = [nc.dram_tensor(f"epat{di}", [128, 128], bf16,
                          kind="ExternalInput") for di in range(3)]
    hv_d = nc.dram_tensor("hv4", [8, 128], bf16, kind="ExternalInput")
    gma_d = nc.dram_tensor("gmA", [128, 512], bf16, kind="ExternalInput")
    gmb_d = nc.dram_tensor("gmB", [128, 512], bf16, kind="ExternalInput")
    a64_d = nc.dram_tensor("a64", [64, 72], bf16, kind="ExternalInput")
    b_d = nc.dram_tensor("b72", [72, 1], f32, kind="ExternalInput")
    r9_d = nc.dram_tensor("r9", [72, 9], bf16, kind="ExternalInput")
    g_d = nc.dram_tensor("g728", [72, 8], bf16, kind="ExternalInput")
    low_d = nc.dram_tensor("low", [H, IC, W], bf16, kind="ExternalOutput")
    high_d = nc.dram_tensor("high", [H, IC, W], bf16, kind="ExternalOutput")

    xd = x_d.ap().tensor

    def flat_rows_ap(r0, np_):
        """np_ partitions <- contiguous chan-blocks starting at row r0
        (partition stride 1536 = one chan-block; 8 blocks per row)."""
        return bass.AP(xd, r0 * ROWSTRIDE, [[CW, np_], [1, CW]])

    def out_ap(dram, t):
        r0, nr = _band_rows(t)
        return bass.AP(dram.ap().tensor, r0 * ROWSTRIDE,
                       [[ROWSTRIDE, nr], [CW, 8], [1, CW]])

    def out_pair_ap(dram, t):
        """Both bands t, t+1 in one descriptor (iter [i][g][band][w],
        matching an SBUF source [112 part][2 bands][1536])."""
        r0, _ = _band_rows(t)
        return bass.AP(dram.ap().tensor, r0 * ROWSTRIDE,
                       [[ROWSTRIDE, BR], [CW, 8], [TSTRIDE, 2], [1, CW]])

    ENGS = None

    with tile.TileContext(nc) as tc, ExitStack() as ctx:
        ENGS = (nc.sync, nc.scalar, nc.gpsimd)
        cpool = ctx.enter_context(tc.tile_pool(name="consts", bufs=1))
        xpool = ctx.enter_context(tc.tile_pool(name="x", bufs=1))
        wpool = ctx.enter_context(tc.tile_pool(name="w", bufs=1))
        spool = ctx.enter_context(tc.tile_pool(name="stage", bufs=4))

        # ---- tiles ----
        xball = xpool.tile([128, NBANDS * BW], bf16)
        xb = [xball[:, t * BW:(t + 1) * BW] for t in range(NBANDS)]
        xv = xball[:].rearrange("p (t b) -> p t b", b=BW)

        warm_s = wpool.tile([128, 128], bf16, name="warm_s")
        warm_m = wpool.tile([128, 512], bf16, name="warm_m")
        nc.vector.memset(warm_s[:], 0.0)
        nc.vector.memset(warm_m[:], 0.0)

        wps_cm = tc.tile_pool(name="wpsum", bufs=1,
                              space=bass.MemorySpace.PSUM)
        wps = wps_cm.__enter__()
        warm_p = wps.tile([128, 512], f32, tag="warm")
        rs_p = wps.tile([64, 192], f32, tag="rowsum")

        # band 13's unloaded partitions: zero BEFORE its loads (NaN in
        # any streamed partition poisons the conv PSUM column; 0*NaN=NaN)
        nc.vector.memset(xb[NBANDS - 1][96:128, PAD:PAD + CW], 0.0)

        # ---- PE warm-up prefix ----
        for _ in range(DUMMY_N):
            nc.tensor.matmul(warm_p[:], warm_s[:], warm_m[:],
                             start=True, stop=True)

        # ---- consts the load/pool phase needs, first on the gpsimd
        # queue (SWDGE issue is ~600ns/descriptor; these are tiny) ----
        e_s = [cpool.tile([128, 128], bf16, name=f"epat{di}")
               for di in range(3)]
        hv_s = cpool.tile([8, 128], bf16)
        gma_s = cpool.tile([128, 512], bf16)
        gmb_s = cpool.tile([128, 512], bf16)
        a64_s = cpool.tile([64, 72], bf16)
        b_s = cpool.tile([72, 1], f32)
        r9_s = cpool.tile([72, 9], bf16)
        g_s = cpool.tile([72, 8], bf16)
        for s, d in ((b_s, b_d), (gma_s, gma_d), (gmb_s, gmb_d)):
            nc.gpsimd.dma_start(s[:], d.ap())

        # ---- x loads: one contiguous descriptor per band, spread over
        # the 3 queues (~120GB/s each on loads).  Band t window holds
        # DRAM rows r0-1..r0+14 at partitions 0..127
        # (p = 8*(row - (r0-1)) + g). ----
        # reflect-row dups must land BEFORE their band's row-sum matmul
        # (which streams all 128 partitions), so issue them ahead of the
        # band mains on their queues.
        nc.sync.dma_start(xb[0][0:8, PAD:PAD + CW], flat_rows_ap(1, 8))
        nc.scalar.dma_start(xb[NBANDS - 1][88:96, PAD:PAD + CW],
                            flat_rows_ap(H - 2, 8))
        for t in range(NBANDS):
            r0, nr = _band_rows(t)
            eng = ENGS[t % 3]
            if t == 0:
                # rows 0..14 -> partitions 8..128; above-halo (reflect
                # row 1) -> partitions 0..8 via the dup above
                eng.dma_start(xb[0][8:128, PAD:PAD + CW],
                              flat_rows_ap(0, 120))
            elif t < NBANDS - 1:
                eng.dma_start(xb[t][0:128, PAD:PAD + CW],
                              flat_rows_ap(r0 - 1, 128))
            else:
                # rows 181..191 -> partitions 0..88; below-halo
                # (reflect row 190) -> partitions 88..96 via the dup
                eng.dma_start(xb[t][0:88, PAD:PAD + CW],
                              flat_rows_ap(r0 - 1, 88))

        # remaining consts (needed from the weight chain onwards)
        for s, d in ((a64_s, a64_d),
                     (e_s[0], e_d[0]), (e_s[1], e_d[1]), (e_s[2], e_d[2]),
                     (hv_s, hv_d), (r9_s, r9_d), (g_s, g_d)):
            nc.gpsimd.dma_start(s[:], d.ap())

        # band-window pads (zero guard columns for the +-1 shifts)
        nc.vector.memset(xv[:, :, 0:PAD], 0.0)
        nc.vector.memset(xv[:, :, PAD + CW:BW], 0.0)

        # pre-load ACT spline tables off the weight-chain critical path
        exp_dummy = wpool.tile([72, 1], f32)
        nc.scalar.activation(exp_dummy[:], b_s[:],
                             mybir.ActivationFunctionType.Exp)

        # ---- pooled row-sums on the PE: rs[8g+cs, w] accumulates the
        # row sums of x[row, 8g+cs, w] across all bands.  Streaming one
        # 192-col chan-sub block per matmul with a per-cs group mask
        # puts each (g, cs) on its own PSUM partition, so the final
        # w-reduce is a tiny [64,192] op and the logit fold is ONE
        # matmul against a64. ----
        for t in range(NBANDS):
            gm = gma_s if t < NBANDS - 1 else gmb_s
            for cs in range(8):
                nc.tensor.matmul(rs_p[:],
                                 gm[:, 64 * cs:64 * (cs + 1)],
                                 xb[t][:, PAD + W * cs:PAD + W * (cs + 1)],
                                 start=(t == 0 and cs == 0),
                                 stop=(t == NBANDS - 1 and cs == 7))

        # ---- weight generation chain ----
        def warm(k, cols=256):
            for _ in range(k):
                nc.tensor.matmul(warm_p[:, 0:cols], warm_s[:],
                                 warm_m[:, 0:cols], start=True, stop=True)

        # w-reduce: [64, 192] PSUM -> [64, 1]; then ONE fold matmul
        p64f = wpool.tile([64, 1], f32)
        nc.vector.tensor_reduce(p64f[:], rs_p[:],
                                axis=mybir.AxisListType.X,
                                op=mybir.AluOpType.add)
        p64s = wpool.tile([64, 1], bf16)
        nc.scalar.copy(p64s[:], p64f[:])
        warm(3)
        # logits: lf[oc] = sum_m a64[m, oc] * P64[m],  m = 8g + cs
        lf_p = wps.tile([72, 1], f32, tag="lf")
        nc.tensor.matmul(lf_p[:], a64_s[:], p64s[:])
        warm(2)
        e72 = wpool.tile([72, 1], f32)
        nc.scalar.activation(e72[:], lf_p[:],
                             mybir.ActivationFunctionType.Exp,
                             bias=b_s[:, 0:1], scale=1.0)
        rhsw = wpool.tile([72, 9], bf16)
        nc.vector.tensor_scalar_mul(rhsw[:], r9_s[:], e72[:, 0:1])
        w89_p = wps.tile([8, 9], f32, tag="w89")
        nc.tensor.matmul(w89_p[:], g_s[:], rhsw[:])
        warm(3)
        s8 = wpool.tile([8, 1], f32)
        nc.vector.tensor_reduce(s8[:], w89_p[:],
                                axis=mybir.AxisListType.X,
                                op=mybir.AluOpType.add)
        r8 = wpool.tile([8, 1], f32)
        nc.vector.reciprocal(r8[:], s8[:])
        w89s = wpool.tile([8, 9], bf16)
        nc.vector.tensor_scalar_mul(w89s[:], w89_p[:], r8[:, 0:1])
        wbig_p = wps.tile([128, 9], f32, tag="wbig")
        nc.tensor.matmul(wbig_p[:], hv_s[:], w89s[:])
        warm(6)
        wsc = wpool.tile([128, 9], f32)
        nc.scalar.copy(wsc[:], wbig_p[:])
        wps_cm.__exit__(None, None, None)

        # ---- the 3 block-banded stationaries: S_dj = sum_di E_di *
        # w[g(q), 3*di+dj] (per-partition row scaling; g(q) = q%8).
        # Each S needs 3 serial ops; split the three chains between DVE
        # and gpsimd so S0 (conv band 0's first stationary) lands fast.
        S = [wpool.tile([128, 128], bf16, name=f"S{dj}") for dj in range(3)]
        for dj in range(3):
            nc.vector.tensor_scalar_mul(S[dj][:], e_s[0][:],
                                        wsc[:, dj:dj + 1])
            for di in (1, 2):
                nc.vector.scalar_tensor_tensor(
                    S[dj][:], e_s[di][:], wsc[:, 3 * di + dj:3 * di + dj + 1],
                    S[dj][:],
                    op0=mybir.AluOpType.mult, op1=mybir.AluOpType.add)
        # edge-fix stationary: with reflection, low[c,0] = S02.x[c,1] +
        # S1.x[c,0] and low[c,191] = S02.x[c,191-1] + S1.x[c,191]
        s02 = wpool.tile([128, 128], bf16, name="s02")
        nc.vector.tensor_tensor(s02[:], S[0][:], S[2][:],
                                op=mybir.AluOpType.add)

        # ---- main loop: one band at a time, acc = 4 PSUM banks
        # (3 x 512 main + edge-fix columns in bank 3) ----
        # conv PSUM: six 1-bank chunk accumulators + one edge tile; each
        # 512-col chunk finishes its 3 dj-passes and drains immediately,
        # so the PE never waits on a whole-band drain chain
        mpool = ctx.enter_context(
            tc.tile_pool(name="mpsum", bufs=6, space=bass.MemorySpace.PSUM))
        epool = ctx.enter_context(
            tc.tile_pool(name="epsum", bufs=1, space=bass.MemorySpace.PSUM))
        # ALL bands' w-edge columns in one pair of 224-col matmuls
        # (xt strides: [band 1540][c 192][e in {1,190} or {0,191}])
        eacc = epool.tile([128, 16 * NBANDS], f32, tag="edge")
        xtc = xv[:, :, PAD:PAD + CW].rearrange("p t (c w) -> p t c w", w=W)
        nc.tensor.matmul(eacc[:], s02[:], xtc[:, :, :, 1:191:189],
                         start=True, stop=False)
        nc.tensor.matmul(eacc[:], S[1][:], xtc[:, :, :, 0:192:191],
                         start=False, stop=True)
        # store queue schedule: 2 stores/band over 3 queues; keep the
        # final bands off the laggy gpsimd software queue so the kernel
        # tail is not gated by it
        sq_rot = [(2, 1), (0, 2), (1, 0)]  # (low_q, high_q) cycling
        for t in range(NBANDS):
            _, nr = _band_rows(t)
            np_ = 8 * nr
            half = t % 2
            lo, hi = half * CW, half * CW + CW
            if half == 0:
                low_st = spool.tile([128, 2 * CW], bf16, tag="low",
                                    name=f"low{t}")
                high_st = spool.tile([128, 2 * CW], bf16, tag="high",
                                     name=f"high{t}")
            for ch in range(3):
                a = mpool.tile([128, 512], f32, tag="acc",
                               name=f"acc{t}_{ch}")
                for j in range(3):
                    off = PAD + CH * ch + j - 1
                    nc.tensor.matmul(a[:], S[j][:], xb[t][:, off:off + CH],
                                     start=(j == 0), stop=(j == 2))
                if ch < 2:
                    nc.scalar.copy(low_st[:, lo + CH * ch:lo + CH * (ch + 1)],
                                   a[:])
                else:
                    nc.vector.tensor_scalar_mul(
                        low_st[:, lo + 1024:hi], a[:], 1.0)
            # output rows of band t live at partitions 8..8+np_
            nc.scalar.copy(
                low_st[:, lo:hi].rearrange("p (c w) -> p c w",
                                           w=W)[:, :, 0:W:W - 1],
                eacc[:, 16 * t:16 * (t + 1)].rearrange(
                    "p (c e) -> p c e", e=2))
            nc.gpsimd.tensor_tensor(high_st[:, lo:lo + 704],
                                    xb[t][:, PAD:PAD + 704],
                                    low_st[:, lo:lo + 704],
                                    op=mybir.AluOpType.subtract)
            nc.vector.tensor_tensor(high_st[:, lo + 704:hi],
                                    xb[t][:, PAD + 704:PAD + CW],
                                    low_st[:, lo + 704:hi],
                                    op=mybir.AluOpType.subtract)
            if t % 2 == 1 and t < NBANDS - 1:
                ql, qh = sq_rot[(t // 2) % 3]
                ENGS[ql].dma_start(out_pair_ap(low_d, t - 1),
                                   low_st[8:120, :])
                ENGS[qh].dma_start(out_pair_ap(high_d, t - 1),
                                   high_st[8:120, :])
            elif t == NBANDS - 2:
                nc.scalar.dma_start(out_ap(low_d, 12),
                                    low_st[8:120, 0:CW])
                nc.sync.dma_start(out_ap(high_d, 12),
                                  high_st[8:120, 0:CW])
            elif t == NBANDS - 1:
                nc.sync.dma_start(out_ap(low_d, 13),
                                  low_st[8:8 + np_, CW:2 * CW])
                nc.scalar.dma_start(out_ap(high_d, 13),
                                    high_st[8:8 + np_, CW:2 * CW])

    nc.compile()
    return nc


def _enable_ldw_opt():
    """Let walrus dedup redundant LDWEIGHTS (same stationary reused by
    consecutive matmuls pipelines at ~N cycles instead of N + ldw)."""
    import concourse.bass_utils as BU
    if getattr(BU, "_ldw_patched", False):
        return
    orig = BU.run_command

    def patched(cmd, *a, **kw):
        cmd = [c.replace("--enable-ldw-opt=false", "--enable-ldw-opt=true")
               if isinstance(c, str) else c for c in cmd]
        return orig(cmd, *a, **kw)

    BU.run_command = patched
    BU._ldw_patched = True


_nc_cache = None


def _get_program():
    global _nc_cache
    if _nc_cache is None:
        # NOTE: _enable_ldw_opt() crashes walrus codegen
        # (visitInstLdweights) with 16-bit stationaries; leave off.
        _nc_cache = _build_program()
    return _nc_cache


def _host_consts(conv_w, bn_gamma, bn_beta, bn_mean, bn_var):
    s_a = bn_gamma / np.sqrt(bn_var + BN_EPS)
    b72 = (bn_beta - bn_mean * s_a).astype(np.float32).reshape(72, 1)
    A = (conv_w * s_a[:, None]) / np.float32(H * W)   # (72, 64)

    # E wiring: band rows r0-1..r0+14 sit at partitions 8*(i_src+1)+g;
    # output row i lives at p = 8*(i+1)+g too (conv output partitions
    # align with input partitions for the DVE subtract), so the source
    # of p for vertical tap di is simply q = p + 8*(di-1).
    epat = [np.zeros((128, 128), np.float32) for _ in range(3)]
    for p in range(8, 120):
        for di in range(3):
            epat[di][p + 8 * (di - 1), p] = 1.0
    epat = [e.astype(ml_dtypes.bfloat16) for e in epat]

    hv4 = (np.arange(8)[:, None] == (np.arange(128)[None, :] % 8)
           ).astype(ml_dtypes.bfloat16)
    # row-sum group masks [128, 8*64]: block cs maps source partition q
    # (valid band row, group g) onto output partition m = 8g + cs
    q = np.arange(128)
    gmA = np.zeros((128, 512), np.float32)
    gmB = np.zeros((128, 512), np.float32)
    for cs in range(8):
        for g in range(8):
            m = 8 * g + cs
            sel = (q % 8 == g)
            gmA[:, 64 * cs + m] = sel & (q >= 8) & (q < 120)
            gmB[:, 64 * cs + m] = sel & (q >= 8) & (q < 88)
    gmA = gmA.astype(ml_dtypes.bfloat16)
    gmB = gmB.astype(ml_dtypes.bfloat16)
    # a64[8g + cs, oc] = A[oc, 8g + cs]  (A cols are channels 8g + cs)
    a64 = np.ascontiguousarray(A.T).astype(ml_dtypes.bfloat16)
    oc = np.arange(72)
    r9 = (oc[:, None] % 9 == np.arange(9)[None, :]
          ).astype(ml_dtypes.bfloat16)
    g728 = (oc[:, None] // 9 == np.arange(8)[None, :]
            ).astype(ml_dtypes.bfloat16)
    return dict(epat0=epat[0], epat1=epat[1], epat2=epat[2], hv4=hv4,
                gmA=gmA, gmB=gmB, a64=a64, b72=b72, r9=r9, g728=g728)


def _prep_inputs(x, conv_w, bn_gamma, bn_beta, bn_mean, bn_var):
    x = np.asarray(x, np.float32)
    consts = _host_consts(np.asarray(conv_w, np.float32),
                          np.asarray(bn_gamma, np.float32),
                          np.asarray(bn_beta, np.float32),
                          np.asarray(bn_mean, np.float32),
                          np.asarray(bn_var, np.float32))
    maps = []
    for i in range(N):
        xr = np.ascontiguousarray(np.transpose(x[i], (1, 0, 2))
                                  ).astype(ml_dtypes.bfloat16)
        maps.append(dict(x=xr, **consts))
    return maps


def _gather(res):
    low = np.stack([np.transpose(np.asarray(res[i]["low"]), (1, 0, 2))
                    for i in range(N)]).astype(np.float32)
    high = np.stack([np.transpose(np.asarray(res[i]["high"]), (1, 0, 2))
                     for i in range(N)]).astype(np.float32)
    return low, high


def kernel(x, conv_w, bn_gamma, bn_beta, bn_mean, bn_var):
    in_maps = _prep_inputs(x, conv_w, bn_gamma, bn_beta, bn_mean, bn_var)
    nc = _get_program()
    res = run_bass_kernel_spmd(nc, in_maps, list(range(N))).results
    return _gather(res)


if __name__ == "__main__":
    rng = np.random.default_rng(0)
    demo = dict(
        x=rng.standard_normal((N, IC, H, W), dtype=np.float32),
        conv_w=rng.standard_normal((72, 64)).astype(np.float32),
        bn_gamma=np.ones(72, np.float32),
        bn_beta=np.zeros(72, np.float32),
        bn_mean=rng.standard_normal(72).astype(np.float32) * 0.1,
        bn_var=rng.uniform(0.5, 1.5, 72).astype(np.float32),
    )
    low, high = kernel(**demo)
    print("ok", low.shape, high.shape)
